# revision 25
# baseline (speedup 1.0000x reference)
"""CrossModalCenterLoss on 8 Trainium2 NeuronCores.

The reference masks the [B, C] distance matrix down to the label-matching
column per row BEFORE clamping, so the loss is exactly

    loss = (sum_b clip(||x_b - centers[labels_b]||^2, 1e-12, 1e12)) / B
         + (C - 1) * 1e-12

No [B, C] matmul is needed — just a gather and a fused squared-distance
reduction. Data-parallel over batch: each of the 8 cores handles 512 rows,
gathers its 512 center rows on-device via indirect DMA (centers stay in
DRAM, replicated), computes the per-core partial sum, and the host
all-reduces the 8 partials into the scalar loss.

Layout: partition p of a core's tiles holds that core's rows 4p..4p+3, so
the labels DMA and the x DMA are fully contiguous. The gather runs as 4
indirect DMAs of 128 rows (the HW consumes exactly one offset per
partition per indirect DMA); the DVE subtract/square for block k is
pipelined behind the issue of gathers k+1..3. A gpsimd partition_all_reduce
collapses the [128,1] row sums so the output store is a single 4-byte
descriptor (a partition-strided store pays ~9 us in write-ack latency).

Raw bacc (no Tile) with manual semaphores: the Tile scheduler's epilogue
(drain + butterfly + bulk semaphore resets) costs several microseconds on
a kernel this small.
"""

import numpy as np

_N_CORES = 8
_B = 4096
_D = 256
_C = 10000
_ROWS = _B // _N_CORES  # 512 rows per core
_P = 128
_K = _ROWS // _P  # 4 rows per partition
_CLAMP_MIN = 1e-12
_CLAMP_MAX = 1e12

_compiled = None


def _build():
    import concourse.bass as bass
    import concourse.mybir as mybir
    from concourse import bacc

    nc = bacc.Bacc(
        "TRN2",
        target_bir_lowering=False,
        debug=False,
        num_devices=_N_CORES,
        enable_partition_id=False,
    )
    x = nc.declare_dram_parameter("x", [_ROWS, _D], mybir.dt.float32, isOutput=False)
    labels = nc.declare_dram_parameter(
        "labels", [_ROWS], mybir.dt.int32, isOutput=False
    )
    centers = nc.declare_dram_parameter(
        "centers", [_C, _D], mybir.dt.float32, isOutput=False
    )
    out = nc.declare_dram_parameter("out", [1, 1], mybir.dt.float32, isOutput=True)

    F = _K * _D  # 1024 free elements per partition

    ones = nc.const_aps.aps[(mybir.dt.float32, 1.0)]  # [128, 1], preamble-initialized

    from contextlib import ExitStack

    with ExitStack() as ctx:
        lab = ctx.enter_context(nc.sbuf_tensor([_P, _K], mybir.dt.int32))
        xt = ctx.enter_context(nc.sbuf_tensor([_P, F], mybir.dt.float32))
        gt = ctx.enter_context(nc.sbuf_tensor([_P, F], mybir.dt.float32))
        sq = ctx.enter_context(nc.sbuf_tensor([_P, F], mybir.dt.float32))
        part_a = ctx.enter_context(nc.sbuf_tensor([_P, 1], mybir.dt.float32))
        part_b = ctx.enter_context(nc.sbuf_tensor([_P, 1], mybir.dt.float32))
        part_c = ctx.enter_context(nc.sbuf_tensor([_P, 1], mybir.dt.float32))
        red = ctx.enter_context(nc.sbuf_tensor([1, 1], mybir.dt.float32))
        psum = ctx.enter_context(nc.psum_tensor([1, 1], mybir.dt.float32))
        names = ["sem_l0", "sem_l1", "sem_l2", "sem_l3", "sem_x",
                 "sem_g0", "sem_g1", "sem_g2", "sem_g3",
                 "sem_v", "sem_m", "sem_r", "sem_d"]
        (sem_l0, sem_l1, sem_l2, sem_l3, sem_x,
         sem_g0, sem_g1, sem_g2, sem_g3,
         sem_v, sem_m, sem_r, sem_d) = [
            ctx.enter_context(nc.semaphore(n)) for n in names
        ]
        block = ctx.enter_context(nc.Block())

        sem_g = [sem_g0, sem_g1, sem_g2, sem_g3]
        sem_l = [sem_l0, sem_l1, sem_l2, sem_l3]
        all_sems = (
            sem_l0, sem_l1, sem_l2, sem_l3, sem_x,
            sem_g0, sem_g1, sem_g2, sem_g3, sem_v, sem_m, sem_r,
        )

        @block.sync
        def _(sync):
            # k-major layout: lab[p, k] = labels[k*128 + p], and
            # xt[p, k*256 + d] = x[k*128 + p, d]. Labels land in 4 chunks so
            # gather 0 starts as soon as its own 512 B of indices is acked
            # instead of waiting for the whole labels transfer.
            labels_ap = labels[:]
            for k in range(_K):
                sync.dma_start(
                    out=lab[:, k : k + 1],
                    in_=labels_ap[k * _P : (k + 1) * _P, None],
                ).then_inc(sem_l[k], 16)
            sync.dma_start(
                out=xt[:].rearrange("p (k d) -> p k d", d=_D),
                in_=x[:].rearrange("(k p) d -> p k d", p=_P),
            ).then_inc(sem_x, 16)
            # Store the scalar once it has been copied out of PSUM.
            sync.wait_ge(sem_r, 1)
            sync.dma_start(out=out[:], in_=red[:]).then_inc(sem_d, 16)
            # Reset sems for NEFF re-execution while the store is in flight.
            for s in all_sems:
                sync.sem_clear(s)

        @block.scalar
        def _(scalar):
            # The NEFF must not complete before the output write is acked.
            # Parking this wait on the otherwise-idle Scalar engine lets the
            # Block-exit barrier work overlap the ~1 us DMA write ack.
            scalar.wait_ge(sem_d, 16)
            scalar.sem_clear(sem_d)

        @block.gpsimd
        def _(gpsimd):
            for k in range(_K):
                gpsimd.wait_ge(sem_l[k], 16)
                # gt[p, k*256:(k+1)*256] = centers[lab[p, k], :]
                gpsimd.indirect_dma_start(
                    out=gt[:, k * _D : (k + 1) * _D],
                    out_offset=None,
                    in_=centers[:],
                    in_offset=bass.IndirectOffsetOnAxis(
                        ap=lab[:, k : k + 1], axis=0
                    ),
                ).then_inc(sem_g[k], 16)

        @block.vector
        def _(vector):
            parts = [part_a, part_b, part_c]
            vector.wait_ge(sem_x, 16)
            for k in range(_K):
                blk = slice(k * _D, (k + 1) * _D)
                vector.wait_ge(sem_g[k], 16)
                vector.tensor_tensor(
                    out=sq[:, blk],
                    in0=xt[:, blk],
                    in1=gt[:, blk],
                    op=mybir.AluOpType.subtract,
                )
                vector.tensor_tensor(
                    out=sq[:, blk],
                    in0=sq[:, blk],
                    in1=sq[:, blk],
                    op=mybir.AluOpType.mult,
                )
                if k == 1:
                    # Row-sum of blocks 0..1 while gathers 2 and 3 stream in.
                    vector.tensor_reduce(
                        out=part_a[:],
                        in_=sq[:, : 2 * _D],
                        axis=mybir.AxisListType.X,
                        op=mybir.AluOpType.add,
                    )
                elif k >= 2:
                    vector.tensor_reduce(
                        out=parts[k - 1][:],
                        in_=sq[:, blk],
                        axis=mybir.AxisListType.X,
                        op=mybir.AluOpType.add,
                    )
            # Drain before signaling: a reduce writes its [128,1] output at
            # the END of the instruction, and a consumer reading within
            # ~100 ns (same engine or cross-engine) sees the stale value.
            vector.drain().then_inc(sem_v, 1)
            # PSUM -> SBUF after the PE cross-partition sum (DMA cannot read
            # PSUM directly).
            vector.wait_ge(sem_m, 1)
            vector.tensor_copy(out=red[:], in_=psum[:])
            vector.drain().then_inc(sem_r, 1)

        @block.tensor
        def _(tensor):
            # Cross-partition sum via PE, accumulating the three row-sum
            # pieces: psum[1,1] = ones.T @ (part_a + part_b + part_c)
            tensor.wait_ge(sem_v, 1)
            tensor.matmul(psum[:], ones, part_a[:], start=True, stop=False)
            tensor.matmul(psum[:], ones, part_b[:], start=False, stop=False)
            tensor.matmul(
                psum[:], ones, part_c[:], start=False, stop=True
            ).then_inc(sem_m, 1)

    nc.compile()
    return nc


def _get_compiled():
    global _compiled
    if _compiled is None:
        _compiled = _build()
    return _compiled


def kernel(x, labels, centers):
    from concourse.bass_utils import run_bass_kernel_spmd

    x = np.ascontiguousarray(np.asarray(x, dtype=np.float32))
    labels_np = np.ascontiguousarray(np.asarray(labels).astype(np.int32))
    centers = np.ascontiguousarray(np.asarray(centers, dtype=np.float32))
    assert x.shape == (_B, _D) and labels_np.shape == (_B,)
    assert centers.shape == (_C, _D)

    nc = _get_compiled()
    in_maps = [
        {
            "x": np.ascontiguousarray(x[i * _ROWS : (i + 1) * _ROWS]),
            "labels": np.ascontiguousarray(labels_np[i * _ROWS : (i + 1) * _ROWS]),
            "centers": centers,
        }
        for i in range(_N_CORES)
    ]
    res = run_bass_kernel_spmd(nc, in_maps, list(range(_N_CORES)))

    # Host-side all-reduce of the per-core partials. Each row's squared
    # distance is hundreds for any non-degenerate input, so the per-element
    # clamp in the reference is a no-op on the selected entries; the (C-1)
    # masked-out zeros per row each clamp up to CLAMP_MIN.
    total = 0.0
    for i in range(_N_CORES):
        total += float(np.asarray(res.results[i]["out"], dtype=np.float64).sum())
    loss = total / _B + (_C - 1) * _CLAMP_MIN
    return np.asarray(loss, dtype=np.float32)


# revision 27
# speedup vs baseline: 1.0656x; 1.0656x over previous
"""CrossModalCenterLoss on 8 Trainium2 NeuronCores.

The reference masks the [B, C] distance matrix down to the label-matching
column per row BEFORE clamping, so the loss is exactly

    loss = (sum_b clip(||x_b - centers[labels_b]||^2, 1e-12, 1e12)) / B
         + (C - 1) * 1e-12

No [B, C] matmul is needed — just a gather and a fused squared-distance
reduction. Data-parallel over batch: each of the 8 cores handles 512 rows,
gathers its 512 center rows on-device via indirect DMA (centers stay in
DRAM, replicated), computes the per-core partial sum, and the host
all-reduces the 8 partials into the scalar loss.

Layout: partition p of a core's tiles holds that core's rows 4p..4p+3, so
the labels DMA and the x DMA are fully contiguous. The gather runs as 4
indirect DMAs of 128 rows (the HW consumes exactly one offset per
partition per indirect DMA); the DVE subtract/square for block k is
pipelined behind the issue of gathers k+1..3. A gpsimd partition_all_reduce
collapses the [128,1] row sums so the output store is a single 4-byte
descriptor (a partition-strided store pays ~9 us in write-ack latency).

Raw bacc (no Tile) with manual semaphores: the Tile scheduler's epilogue
(drain + butterfly + bulk semaphore resets) costs several microseconds on
a kernel this small.
"""

import numpy as np

_N_CORES = 8
_B = 4096
_D = 256
_C = 10000
_ROWS = _B // _N_CORES  # 512 rows per core
_P = 128
_K = _ROWS // _P  # 4 rows per partition
_CLAMP_MIN = 1e-12
_CLAMP_MAX = 1e12

_compiled = None


def _build():
    import concourse.bass as bass
    import concourse.mybir as mybir
    from concourse import bacc

    nc = bacc.Bacc(
        "TRN2",
        target_bir_lowering=False,
        debug=False,
        num_devices=_N_CORES,
        enable_partition_id=False,
    )
    x = nc.declare_dram_parameter("x", [_ROWS, _D], mybir.dt.float32, isOutput=False)
    labels = nc.declare_dram_parameter(
        "labels", [_ROWS], mybir.dt.int32, isOutput=False
    )
    centers = nc.declare_dram_parameter(
        "centers", [_C, _D], mybir.dt.float32, isOutput=False
    )
    out = nc.declare_dram_parameter("out", [1, 1], mybir.dt.float32, isOutput=True)

    F = _K * _D  # 1024 free elements per partition

    ones = nc.const_aps.aps[(mybir.dt.float32, 1.0)]  # [128, 1], preamble-initialized

    from contextlib import ExitStack

    with ExitStack() as ctx:
        lab = ctx.enter_context(nc.sbuf_tensor([_P, _K], mybir.dt.int32))
        xt = ctx.enter_context(nc.sbuf_tensor([_P, F], mybir.dt.float32))
        gt = ctx.enter_context(nc.sbuf_tensor([_P, F], mybir.dt.float32))
        sq = ctx.enter_context(nc.sbuf_tensor([_P, F], mybir.dt.float32))
        part_a = ctx.enter_context(nc.sbuf_tensor([_P, 1], mybir.dt.float32))
        part_b = ctx.enter_context(nc.sbuf_tensor([_P, 1], mybir.dt.float32))
        part_c = ctx.enter_context(nc.sbuf_tensor([_P, 1], mybir.dt.float32))
        red = ctx.enter_context(nc.sbuf_tensor([1, 1], mybir.dt.float32))
        psum = ctx.enter_context(nc.psum_tensor([1, 1], mybir.dt.float32))
        names = ["sem_l0", "sem_l1", "sem_l2", "sem_l3", "sem_x",
                 "sem_g0", "sem_g1", "sem_g2", "sem_g3",
                 "sem_v", "sem_m", "sem_r", "sem_d"]
        (sem_l0, sem_l1, sem_l2, sem_l3, sem_x,
         sem_g0, sem_g1, sem_g2, sem_g3,
         sem_v, sem_m, sem_r, sem_d) = [
            ctx.enter_context(nc.semaphore(n)) for n in names
        ]
        block = ctx.enter_context(nc.Block())

        sem_g = [sem_g0, sem_g1, sem_g2, sem_g3]
        sem_l = [sem_l0, sem_l1, sem_l2, sem_l3]
        all_sems = (
            sem_l0, sem_l1, sem_l2, sem_l3, sem_x,
            sem_g0, sem_g1, sem_g2, sem_g3, sem_v, sem_m, sem_r,
        )

        @block.sync
        def _(sync):
            # lab[p, k] = labels[4p + k]; xt[p, k*256 + d] = x[4p + k, d].
            # One DMA each: issue slots on the HWDGE queue cost ~0.6 us, so
            # chunking the tiny labels transfer is a net loss.
            sync.dma_start(
                out=lab[:], in_=labels[:].rearrange("(p k) -> p k", p=_P)
            ).then_inc(sem_l[0], 16)
            sync.dma_start(
                out=xt[:], in_=x[:].rearrange("(p k) d -> p (k d)", p=_P)
            ).then_inc(sem_x, 16)
            # Store the scalar once it has been copied out of PSUM.
            sync.wait_ge(sem_r, 1)
            sync.dma_start(out=out[:], in_=red[:]).then_inc(sem_d, 16)
            # Reset sems for NEFF re-execution while the store is in flight.
            for s in all_sems:
                sync.sem_clear(s)

        @block.scalar
        def _(scalar):
            # The NEFF must not complete before the output write is acked.
            # Parking this wait on the otherwise-idle Scalar engine lets the
            # Block-exit barrier work overlap the ~1 us DMA write ack.
            scalar.wait_ge(sem_d, 16)
            scalar.sem_clear(sem_d)

        @block.gpsimd
        def _(gpsimd):
            gpsimd.wait_ge(sem_l[0], 16)
            for k in range(_K):
                # gt[p, k*256:(k+1)*256] = centers[lab[p, k], :]
                gpsimd.indirect_dma_start(
                    out=gt[:, k * _D : (k + 1) * _D],
                    out_offset=None,
                    in_=centers[:],
                    in_offset=bass.IndirectOffsetOnAxis(
                        ap=lab[:, k : k + 1], axis=0
                    ),
                ).then_inc(sem_g[k], 16)

        @block.vector
        def _(vector):
            parts = [part_a, part_b, part_c]
            vector.wait_ge(sem_x, 16)
            for k in range(_K):
                blk = slice(k * _D, (k + 1) * _D)
                vector.wait_ge(sem_g[k], 16)
                vector.tensor_tensor(
                    out=sq[:, blk],
                    in0=xt[:, blk],
                    in1=gt[:, blk],
                    op=mybir.AluOpType.subtract,
                )
                vector.tensor_tensor(
                    out=sq[:, blk],
                    in0=sq[:, blk],
                    in1=sq[:, blk],
                    op=mybir.AluOpType.mult,
                )
                if k == 1:
                    # Row-sum of blocks 0..1 while gathers 2 and 3 stream in.
                    vector.tensor_reduce(
                        out=part_a[:],
                        in_=sq[:, : 2 * _D],
                        axis=mybir.AxisListType.X,
                        op=mybir.AluOpType.add,
                    )
                elif k >= 2:
                    vector.tensor_reduce(
                        out=parts[k - 1][:],
                        in_=sq[:, blk],
                        axis=mybir.AxisListType.X,
                        op=mybir.AluOpType.add,
                    )
            # Drain before signaling: a reduce writes its [128,1] output at
            # the END of the instruction, and a consumer reading within
            # ~100 ns (same engine or cross-engine) sees the stale value.
            vector.drain().then_inc(sem_v, 1)
            # PSUM -> SBUF after the PE cross-partition sum (DMA cannot read
            # PSUM directly).
            vector.wait_ge(sem_m, 1)
            vector.tensor_copy(out=red[:], in_=psum[:])
            vector.drain().then_inc(sem_r, 1)

        @block.tensor
        def _(tensor):
            # Cross-partition sum via PE, accumulating the three row-sum
            # pieces: psum[1,1] = ones.T @ (part_a + part_b + part_c)
            tensor.wait_ge(sem_v, 1)
            tensor.matmul(psum[:], ones, part_a[:], start=True, stop=False)
            tensor.matmul(psum[:], ones, part_b[:], start=False, stop=False)
            tensor.matmul(
                psum[:], ones, part_c[:], start=False, stop=True
            ).then_inc(sem_m, 1)

    nc.compile()
    return nc


def _get_compiled():
    global _compiled
    if _compiled is None:
        _compiled = _build()
    return _compiled


def kernel(x, labels, centers):
    from concourse.bass_utils import run_bass_kernel_spmd

    x = np.ascontiguousarray(np.asarray(x, dtype=np.float32))
    labels_np = np.ascontiguousarray(np.asarray(labels).astype(np.int32))
    centers = np.ascontiguousarray(np.asarray(centers, dtype=np.float32))
    assert x.shape == (_B, _D) and labels_np.shape == (_B,)
    assert centers.shape == (_C, _D)

    nc = _get_compiled()
    in_maps = [
        {
            "x": np.ascontiguousarray(x[i * _ROWS : (i + 1) * _ROWS]),
            "labels": np.ascontiguousarray(labels_np[i * _ROWS : (i + 1) * _ROWS]),
            "centers": centers,
        }
        for i in range(_N_CORES)
    ]
    res = run_bass_kernel_spmd(nc, in_maps, list(range(_N_CORES)))

    # Host-side all-reduce of the per-core partials. Each row's squared
    # distance is hundreds for any non-degenerate input, so the per-element
    # clamp in the reference is a no-op on the selected entries; the (C-1)
    # masked-out zeros per row each clamp up to CLAMP_MIN.
    total = 0.0
    for i in range(_N_CORES):
        total += float(np.asarray(res.results[i]["out"], dtype=np.float64).sum())
    loss = total / _B + (_C - 1) * _CLAMP_MIN
    return np.asarray(loss, dtype=np.float32)


# revision 30
# speedup vs baseline: 1.0865x; 1.0196x over previous
"""CrossModalCenterLoss on 8 Trainium2 NeuronCores.

The reference masks the [B, C] distance matrix down to the label-matching
column per row BEFORE clamping, so the loss is exactly

    loss = (sum_b clip(||x_b - centers[labels_b]||^2, 1e-12, 1e12)) / B
         + (C - 1) * 1e-12

No [B, C] matmul is needed — just a gather and a fused squared-distance
reduction. Data-parallel over batch: each of the 8 cores handles 512 rows,
gathers its 512 center rows on-device via indirect DMA (centers stay in
DRAM, replicated), computes the per-core partial sum, and the host
all-reduces the 8 partials into the scalar loss.

Layout: partition p of a core's tiles holds that core's rows 4p..4p+3, so
the labels DMA and the x DMA are fully contiguous. The gather runs as 4
indirect DMAs of 128 rows (the HW consumes exactly one offset per
partition per indirect DMA); the DVE subtract/square/row-sum for block k
is pipelined behind the later gathers' transfers. The PE collapses the
[128,1] row-sum partials across partitions via PSUM-accumulating matmuls
against the preamble's const-1.0 column, so the output store is a single
4-byte descriptor (a partition-strided store pays ~9 us in write-ack
latency; a gpsimd custom-op reduce pays ~6 us of ucode library load).

Raw bacc (no Tile) with manual semaphores: the Tile scheduler's epilogue
(drain + butterfly + bulk semaphore resets) costs several microseconds on
a kernel this small.
"""

import numpy as np

_N_CORES = 8
_B = 4096
_D = 256
_C = 10000
_ROWS = _B // _N_CORES  # 512 rows per core
_P = 128
_K = _ROWS // _P  # 4 rows per partition
_CLAMP_MIN = 1e-12
_CLAMP_MAX = 1e12

_compiled = None


def _build():
    import concourse.bass as bass
    import concourse.mybir as mybir
    from concourse import bacc

    nc = bacc.Bacc(
        "TRN2",
        target_bir_lowering=False,
        debug=False,
        num_devices=_N_CORES,
        enable_partition_id=False,
        num_swdge_queues=2,
    )
    x = nc.declare_dram_parameter("x", [_ROWS, _D], mybir.dt.float32, isOutput=False)
    labels = nc.declare_dram_parameter(
        "labels", [_ROWS], mybir.dt.int32, isOutput=False
    )
    centers = nc.declare_dram_parameter(
        "centers", [_C, _D], mybir.dt.float32, isOutput=False
    )
    out = nc.declare_dram_parameter("out", [1, 1], mybir.dt.float32, isOutput=True)

    F = _K * _D  # 1024 free elements per partition

    ones = nc.const_aps.aps[(mybir.dt.float32, 1.0)]  # [128, 1], preamble-initialized

    from contextlib import ExitStack

    with ExitStack() as ctx:
        lab = ctx.enter_context(nc.sbuf_tensor([_P, _K], mybir.dt.int32))
        xt = ctx.enter_context(nc.sbuf_tensor([_P, F], mybir.dt.float32))
        gt = ctx.enter_context(nc.sbuf_tensor([_P, F], mybir.dt.float32))
        sq = ctx.enter_context(nc.sbuf_tensor([_P, F], mybir.dt.float32))
        part_a = ctx.enter_context(nc.sbuf_tensor([_P, 1], mybir.dt.float32))
        part_b = ctx.enter_context(nc.sbuf_tensor([_P, 1], mybir.dt.float32))
        part_c = ctx.enter_context(nc.sbuf_tensor([_P, 1], mybir.dt.float32))
        red = ctx.enter_context(nc.sbuf_tensor([1, 1], mybir.dt.float32))
        psum = ctx.enter_context(nc.psum_tensor([1, 1], mybir.dt.float32))
        names = ["sem_l0", "sem_l1", "sem_l2", "sem_l3", "sem_x",
                 "sem_g0", "sem_g1", "sem_g2", "sem_g3",
                 "sem_v", "sem_m", "sem_r", "sem_d"]
        (sem_l0, sem_l1, sem_l2, sem_l3, sem_x,
         sem_g0, sem_g1, sem_g2, sem_g3,
         sem_v, sem_m, sem_r, sem_d) = [
            ctx.enter_context(nc.semaphore(n)) for n in names
        ]
        block = ctx.enter_context(nc.Block())

        sem_g = [sem_g0, sem_g1, sem_g2, sem_g3]
        sem_l = [sem_l0, sem_l1, sem_l2, sem_l3]
        all_sems = (
            sem_l0, sem_l1, sem_l2, sem_l3, sem_x,
            sem_g0, sem_g1, sem_g2, sem_g3, sem_v, sem_m, sem_r,
        )

        @block.sync
        def _(sync):
            # lab[p, k] = labels[4p + k]; xt[p, k*256 + d] = x[4p + k, d].
            # One DMA each: issue slots on the HWDGE queue cost ~0.6 us, so
            # chunking the tiny labels transfer is a net loss.
            sync.dma_start(
                out=lab[:], in_=labels[:].rearrange("(p k) -> p k", p=_P)
            ).then_inc(sem_l[0], 16)
            sync.dma_start(
                out=xt[:], in_=x[:].rearrange("(p k) d -> p (k d)", p=_P)
            ).then_inc(sem_x, 16)
            # Store the scalar once it has been copied out of PSUM.
            sync.wait_ge(sem_r, 1)
            sync.dma_start(out=out[:], in_=red[:]).then_inc(sem_d, 16)
            # Reset sems for NEFF re-execution while the store is in flight.
            for s in all_sems:
                sync.sem_clear(s)

        @block.scalar
        def _(scalar):
            # The NEFF must not complete before the output write is acked.
            # Parking this wait on the otherwise-idle Scalar engine lets the
            # Block-exit barrier work overlap the ~1 us DMA write ack.
            scalar.wait_ge(sem_d, 16)
            scalar.sem_clear(sem_d)

        @block.gpsimd
        def _(gpsimd):
            gpsimd.wait_ge(sem_l[0], 16)
            for k in range(_K):
                # gt[p, k*256:(k+1)*256] = centers[lab[p, k], :]
                g = gpsimd.indirect_dma_start(
                    out=gt[:, k * _D : (k + 1) * _D],
                    out_offset=None,
                    in_=centers[:],
                    in_offset=bass.IndirectOffsetOnAxis(
                        ap=lab[:, k : k + 1], axis=0
                    ),
                )
                if k % 2:
                    # Alternate the two SWDGE queues so consecutive gathers'
                    # transfers interleave across the SDMA engines instead of
                    # queueing strictly behind each other (the 1 KB random
                    # reads are HBM-latency-bound, not bandwidth-bound).
                    g.ins.queue = "qPoolDynamic1"
                g.then_inc(sem_g[k], 16)

        @block.vector
        def _(vector):
            parts = [part_a, part_b, part_c]
            vector.wait_ge(sem_x, 16)
            for k in range(_K):
                blk = slice(k * _D, (k + 1) * _D)
                vector.wait_ge(sem_g[k], 16)
                vector.tensor_tensor(
                    out=sq[:, blk],
                    in0=xt[:, blk],
                    in1=gt[:, blk],
                    op=mybir.AluOpType.subtract,
                )
                vector.tensor_tensor(
                    out=sq[:, blk],
                    in0=sq[:, blk],
                    in1=sq[:, blk],
                    op=mybir.AluOpType.mult,
                )
                if k == 1:
                    # Row-sum of blocks 0..1 while gathers 2 and 3 stream in.
                    vector.tensor_reduce(
                        out=part_a[:],
                        in_=sq[:, : 2 * _D],
                        axis=mybir.AxisListType.X,
                        op=mybir.AluOpType.add,
                    )
                elif k >= 2:
                    vector.tensor_reduce(
                        out=parts[k - 1][:],
                        in_=sq[:, blk],
                        axis=mybir.AxisListType.X,
                        op=mybir.AluOpType.add,
                    )
            # Drain before signaling: a reduce writes its [128,1] output at
            # the END of the instruction, and a consumer reading within
            # ~100 ns (same engine or cross-engine) sees the stale value.
            vector.drain().then_inc(sem_v, 1)
            # PSUM -> SBUF after the PE cross-partition sum (DMA cannot read
            # PSUM directly).
            vector.wait_ge(sem_m, 1)
            vector.tensor_copy(out=red[:], in_=psum[:])
            vector.drain().then_inc(sem_r, 1)

        @block.tensor
        def _(tensor):
            # Cross-partition sum via PE, accumulating the three row-sum
            # pieces: psum[1,1] = ones.T @ (part_a + part_b + part_c)
            tensor.wait_ge(sem_v, 1)
            tensor.matmul(psum[:], ones, part_a[:], start=True, stop=False)
            tensor.matmul(psum[:], ones, part_b[:], start=False, stop=False)
            tensor.matmul(
                psum[:], ones, part_c[:], start=False, stop=True
            ).then_inc(sem_m, 1)

    nc.compile()
    return nc


def _get_compiled():
    global _compiled
    if _compiled is None:
        _compiled = _build()
    return _compiled


def kernel(x, labels, centers):
    from concourse.bass_utils import run_bass_kernel_spmd

    x = np.ascontiguousarray(np.asarray(x, dtype=np.float32))
    labels_np = np.ascontiguousarray(np.asarray(labels).astype(np.int32))
    centers = np.ascontiguousarray(np.asarray(centers, dtype=np.float32))
    assert x.shape == (_B, _D) and labels_np.shape == (_B,)
    assert centers.shape == (_C, _D)

    nc = _get_compiled()
    in_maps = [
        {
            "x": np.ascontiguousarray(x[i * _ROWS : (i + 1) * _ROWS]),
            "labels": np.ascontiguousarray(labels_np[i * _ROWS : (i + 1) * _ROWS]),
            "centers": centers,
        }
        for i in range(_N_CORES)
    ]
    res = run_bass_kernel_spmd(nc, in_maps, list(range(_N_CORES)))

    # Host-side all-reduce of the per-core partials. Each row's squared
    # distance is hundreds for any non-degenerate input, so the per-element
    # clamp in the reference is a no-op on the selected entries; the (C-1)
    # masked-out zeros per row each clamp up to CLAMP_MIN.
    total = 0.0
    for i in range(_N_CORES):
        total += float(np.asarray(res.results[i]["out"], dtype=np.float64).sum())
    loss = total / _B + (_C - 1) * _CLAMP_MIN
    return np.asarray(loss, dtype=np.float32)
